# revision 32
# baseline (speedup 1.0000x reference)
"""MoE experts kernel for Trainium2 (8 NeuronCores, expert-parallel),
mixed-precision: per (token, expert) pair, low routing weight -> fp8
(e4m3 DoubleRow matmuls, 2x PE rate), high routing weight -> fp16.

Reference computation (per token t, top-k expert e with gate p):
    y[t] = sum_k p[t,k] * down_e @ (silu(x[t] @ gate_e) * (x[t] @ up_e))
with per-expert capacity CAP=1024 (tokens beyond capacity dropped).

Error budget: final tolerance 2e-2. fp8-everything measures 6.6e-2;
routing pairs with p < THETA=0.09 (~35% of pairs, ~4.7% of sum p^2)
through the fp8 path gives ~1.4e-2 end-to-end, while cutting PE work
by ~17% (fp8 DoubleRow contracts 256 rows/instruction at the same
per-column rate as fp16's 128).

Layout trick: a [128, NH, 128] stationary weight tile serves both
paths -- fp16 matmuls slice [:, h, :], fp8 DoubleRow slices
[:, 2hh:2hh+2, :] (the pair dim is just two adjacent h-chunks).

Scales (fp8 path): gate*64 (undone by silu's input scale), up*4,
down*16 -> device output = 64*o, fp16-safe; host divides by 64.
"""

import os
import sys

sys.path.insert(0, "/opt/trn_rl_repo")

import numpy as np
import ml_dtypes

E, H, I, T, K = 64, 2048, 768, 4096, 8
CAP = 1024
NCORES = 8
EPC = E // NCORES  # experts per core
NH = H // 128  # 16 contraction chunks for gate/up
NI = I // 128  # 6 contraction chunks for down

THETA = 0.10  # routing-weight threshold: p < THETA -> fp8 path
SG, SU, SD = 64.0, 4.0, 16.0  # fp8 quantization scales
E4 = ml_dtypes.float8_e4m3

_prog_cache = {}
LAST_EXEC_NS = None
LAST_RESULTS = None


def _groups(npad):
    ng = -(-npad // 512)
    w = -(-npad // ng)
    out = []
    s = 0
    while s < npad:
        e = min(s + w, npad)
        out.append((s, e))
        s = e
    return out


def _build_program(w16s, w8s):
    import concourse.bacc as bacc
    import concourse.mybir as mybir
    from concourse.tile import TileContext

    f32 = mybir.dt.float32
    f16 = mybir.dt.float16
    f8 = mybir.dt.float8e4
    DR = mybir.MatmulPerfMode.DoubleRow
    SILU = mybir.ActivationFunctionType.Silu

    nc = bacc.Bacc(None, target_bir_lowering=False)
    xT16s = [
        nc.declare_dram_parameter(f"x16_{j}", [NH, 128, w], f16, isOutput=False)
        for j, w in enumerate(w16s)
    ]
    xT8s = [
        nc.declare_dram_parameter(f"x8_{j}", [NH, 128, w], f8, isOutput=False)
        for j, w in enumerate(w8s)
    ]
    gw16 = nc.declare_dram_parameter("gw16", [EPC, NI, 128, NH, 128], f16, isOutput=False)
    uw16 = nc.declare_dram_parameter("uw16", [EPC, NI, 128, NH, 128], f16, isOutput=False)
    dw16 = nc.declare_dram_parameter("dw16", [EPC, NH, 128, NI, 128], f16, isOutput=False)
    gw8 = nc.declare_dram_parameter("gw8", [EPC, NI, 128, NH, 128], f8, isOutput=False)
    uw8 = nc.declare_dram_parameter("uw8", [EPC, NI, 128, NH, 128], f8, isOutput=False)
    dw8 = nc.declare_dram_parameter("dw8", [EPC, NH, 128, NI, 128], f8, isOutput=False)
    # [q, 128p, 4hq, w]: element (q, p, hq, n) = O^T[(4q+hq)*128+p... wait see
    # host combine — stored so the device DMA is layout-matched to the ot tile.
    yT16s = [
        nc.declare_dram_parameter(f"y16_{j}", [NH // 4, 128, 4, w], f16, isOutput=True)
        for j, w in enumerate(w16s)
    ]
    yT8s = [
        nc.declare_dram_parameter(f"y8_{j}", [NH // 4, 128, 4, w], f16, isOutput=True)
        for j, w in enumerate(w8s)
    ]

    with TileContext(nc) as tc:
        with (
            tc.sbuf_pool(name="xp", bufs=2) as xp,
            tc.sbuf_pool(name="wp", bufs=3) as wp,
            tc.sbuf_pool(name="hp", bufs=2) as hp,
            tc.sbuf_pool(name="op", bufs=3) as op,
            tc.sbuf_pool(name="tp", bufs=3) as tp,
            tc.psum_pool(name="pp", bufs=2) as pp,
        ):
            for k in range(EPC):
                wa, wb = w16s[k], w8s[k]
                ga16, ga8 = _groups(wa), _groups(wb)
                # Three parallel DMA flows: gate/up weight stream alone on the
                # Sync DGE queue (the big stream, never blocked); down weights
                # early on the Scalar DGE queue (transfer during phase A);
                # x tiles and outputs on the idle GpSimd SWDGE queue.
                # expert 0's x tiles go on the fast Sync HW queue, interleaved
                # with the first gate/up weights so the PE starts ASAP (SWDGE
                # takes ~5us/DMA to spin up descriptor gen — fine once
                # prefetch is a full expert ahead, fatal on startup)
                xr16 = xT16s[k].rearrange("h p n -> p h n")
                xr8 = xT8s[k].rearrange("h p n -> p h n")
                xts16 = []
                xts8 = []
                pre = {}
                if k == 0:
                    # startup fast path, all on the Sync HW queue in the exact
                    # order the PE consumes: first x chunk, first gate/up
                    # weights, remaining x chunks, first fp8 weights. (SWDGE
                    # takes ~5us/DMA to spin up; HW DGE delivers immediately.)
                    xt = xp.tile([128, 4, wa], f16, name="xt16_0", tag="xt16_0")
                    nc.sync.dma_start(out=xt, in_=xr16[:, 0:4, :])
                    xts16.append(xt)
                    g16 = wp.tile([128, NH, 128], f16, name="g16", tag="g16", bufs=3)
                    u16 = wp.tile([128, NH, 128], f16, name="u16", tag="u16", bufs=3)
                    nc.sync.dma_start(out=g16, in_=gw16[k, 0, :, :, :])
                    nc.sync.dma_start(out=u16, in_=uw16[k, 0, :, :, :])
                    for j in range(1, 4):
                        xt = xp.tile([128, 4, wa], f16, name=f"xt16_{j}", tag=f"xt16_{j}")
                        nc.sync.dma_start(out=xt, in_=xr16[:, 4 * j : 4 * (j + 1), :])
                        xts16.append(xt)
                    for j in range(4):
                        xt = xp.tile([128, 4, wb], f8, name=f"xt8_{j}", tag=f"xt8_{j}")
                        nc.sync.dma_start(out=xt, in_=xr8[:, 4 * j : 4 * (j + 1), :])
                        xts8.append(xt)
                    g8 = wp.tile([128, NH, 128], f8, name="g8", tag="g8")
                    u8 = wp.tile([128, NH, 128], f8, name="u8", tag="u8")
                    nc.sync.dma_start(out=g8, in_=gw8[k, 0, :, :, :])
                    nc.sync.dma_start(out=u8, in_=uw8[k, 0, :, :, :])
                    pre[0] = (g16, u16, g8, u8)
                else:
                    for j in range(4):
                        xt = xp.tile([128, 4, wa], f16, name=f"xt16_{j}", tag=f"xt16_{j}")
                        nc.gpsimd.dma_start(out=xt, in_=xr16[:, 4 * j : 4 * (j + 1), :])
                        xts16.append(xt)
                    for j in range(4):
                        xt = xp.tile([128, 4, wb], f8, name=f"xt8_{j}", tag=f"xt8_{j}")
                        nc.gpsimd.dma_start(out=xt, in_=xr8[:, 4 * j : 4 * (j + 1), :])
                        xts8.append(xt)
                d16 = wp.tile([128, NH, NI, 128], f16, name="d16", tag="d16", bufs=2)
                d8 = wp.tile([128, NH, NI, 128], f8, name="d8", tag="d8", bufs=2)
                dr16 = dw16[k].rearrange("h p i m -> p h i m")
                dr8 = dw8[k].rearrange("h p i m -> p h i m")
                if k > 0:
                    # k=0's down-weight DMAs are deferred into the i-loop so
                    # the first silu groups aren't stuck behind them on the
                    # Scalar queue
                    nc.scalar.dma_start(out=d16[:, 0 : NH // 2, :, :], in_=dr16[:, 0 : NH // 2, :, :])
                    nc.scalar.dma_start(out=d16[:, NH // 2 :, :, :], in_=dr16[:, NH // 2 :, :, :])
                    nc.scalar.dma_start(out=d8[:, 0 : NH // 2, :, :], in_=dr8[:, 0 : NH // 2, :, :])
                    nc.scalar.dma_start(out=d8[:, NH // 2 :, :, :], in_=dr8[:, NH // 2 :, :, :])
                hm16 = hp.tile([128, NI, wa], f16, name="hm16", tag="hm16")
                hm8 = hp.tile([128, NI, wb], f8, name="hm8", tag="hm8")
                for i in range(NI):
                    if k == 0 and i == 1:
                        nc.scalar.dma_start(out=d16[:, 0 : NH // 2, :, :], in_=dr16[:, 0 : NH // 2, :, :])
                        nc.scalar.dma_start(out=d16[:, NH // 2 :, :, :], in_=dr16[:, NH // 2 :, :, :])
                        nc.scalar.dma_start(out=d8[:, 0 : NH // 2, :, :], in_=dr8[:, 0 : NH // 2, :, :])
                        nc.scalar.dma_start(out=d8[:, NH // 2 :, :, :], in_=dr8[:, NH // 2 :, :, :])
                    if i in pre:
                        g16, u16, g8, u8 = pre[i]
                    else:
                        g16 = wp.tile([128, NH, 128], f16, name="g16", tag="g16", bufs=3)
                        u16 = wp.tile([128, NH, 128], f16, name="u16", tag="u16", bufs=3)
                        nc.sync.dma_start(out=g16, in_=gw16[k, i, :, :, :])
                        nc.sync.dma_start(out=u16, in_=uw16[k, i, :, :, :])
                        g8 = wp.tile([128, NH, 128], f8, name="g8", tag="g8")
                        u8 = wp.tile([128, NH, 128], f8, name="u8", tag="u8")
                        nc.gpsimd.dma_start(out=g8, in_=gw8[k, i, :, :, :])
                        nc.gpsimd.dma_start(out=u8, in_=uw8[k, i, :, :, :])
                    for g0, g1 in ga16:
                        wdt = g1 - g0
                        psg = pp.tile([128, wdt], f32, name="psg", tag="psg", bufs=3)
                        psu = pp.tile([128, wdt], f32, name="psu", tag="psu", bufs=2)
                        for h in range(NH):
                            nc.tensor.matmul(
                                psg, g16[:, h, :], xts16[h // 4][:, h % 4, g0:g1],
                                start=(h == 0), stop=(h == NH - 1),
                            )
                        for h in range(NH):
                            nc.tensor.matmul(
                                psu, u16[:, h, :], xts16[h // 4][:, h % 4, g0:g1],
                                start=(h == 0), stop=(h == NH - 1),
                            )
                        sil = tp.tile([128, wdt], f32, name="sil", tag="sil")
                        nc.scalar.activation(sil, psg, SILU)
                        nc.vector.tensor_mul(hm16[:, i, g0:g1], sil, psu)
                    for g0, g1 in ga8:
                        wdt = g1 - g0
                        psg8 = pp.tile([128, wdt], f32, name="psg8", tag="psg", bufs=3)
                        psu8 = pp.tile([128, wdt], f32, name="psu8", tag="psu", bufs=2)
                        for hh in range(NH // 2):
                            m = hh % 2
                            nc.tensor.matmul(
                                psg8,
                                g8[:, 2 * hh : 2 * hh + 2, :],
                                xts8[hh // 2][:, 2 * m : 2 * m + 2, g0:g1],
                                start=(hh == 0), stop=(hh == NH // 2 - 1),
                                perf_mode=DR,
                            )
                        for hh in range(NH // 2):
                            m = hh % 2
                            nc.tensor.matmul(
                                psu8,
                                u8[:, 2 * hh : 2 * hh + 2, :],
                                xts8[hh // 2][:, 2 * m : 2 * m + 2, g0:g1],
                                start=(hh == 0), stop=(hh == NH // 2 - 1),
                                perf_mode=DR,
                            )
                        sil8 = tp.tile([128, wdt], f32, name="sil8", tag="sil")
                        nc.scalar.activation(sil8, psg8, SILU, scale=1.0 / SG)
                        nc.vector.tensor_mul(hm8[:, i, g0:g1], sil8, psu8)
                for q in range(NH // 4):
                    ot16 = op.tile([128, 4, wa], f16, name="ot16", tag="ot16")
                    ot8 = op.tile([128, 4, wb], f16, name="ot8", tag="ot8")
                    for hq in range(4):
                        h = 4 * q + hq
                        for g0, g1 in ga16:
                            wdt = g1 - g0
                            pso = pp.tile([128, wdt], f32, name="pso", tag="pso", bufs=3)
                            for i in range(NI):
                                nc.tensor.matmul(
                                    pso, d16[:, h, i, :], hm16[:, i, g0:g1],
                                    start=(i == 0), stop=(i == NI - 1),
                                )
                            nc.vector.tensor_copy(ot16[:, hq, g0:g1], pso)
                        for g0, g1 in ga8:
                            wdt = g1 - g0
                            pso8 = pp.tile([128, wdt], f32, name="pso8", tag="pso", bufs=3)
                            for ii in range(NI // 2):
                                nc.tensor.matmul(
                                    pso8,
                                    d8[:, h, 2 * ii : 2 * ii + 2, :],
                                    hm8[:, 2 * ii : 2 * ii + 2, g0:g1],
                                    start=(ii == 0), stop=(ii == NI // 2 - 1),
                                    perf_mode=DR,
                                )
                            nc.vector.tensor_copy(ot8[:, hq, g0:g1], pso8)
                    nc.gpsimd.dma_start(out=yT16s[k][q, :, :, :], in_=ot16)
                    nc.gpsimd.dma_start(out=yT8s[k][q, :, :, :], in_=ot8)
    nc.compile()
    return nc


def _install_neff_cache():
    """Cache walrus NEFF compiles on disk keyed by BIR hash (compile of the
    ~10k-instruction program takes minutes; the BIR is deterministic)."""
    import hashlib
    import shutil

    import concourse.bass2jax as bass2jax
    from concourse.bass_utils import compile_bir_kernel as _orig

    if getattr(bass2jax.compile_bir_kernel, "_moe_cached", False):
        return
    cache_dir = os.environ.get("BASS_NEFF_CACHE", "/tmp/bass_neff_cache")
    try:
        os.makedirs(cache_dir, exist_ok=True)
    except OSError:
        return

    def cached(bir_json, tmpdir, neff_name="file.neff"):
        key = hashlib.sha256(bir_json).hexdigest()[:24]
        cpath = os.path.join(cache_dir, key + ".neff")
        dst = os.path.join(tmpdir, neff_name)
        if os.path.exists(cpath):
            shutil.copy(cpath, dst)
            return dst
        out = _orig(bir_json, tmpdir, neff_name)
        try:
            shutil.copy(out, cpath)
        except OSError:
            pass
        return out

    cached._moe_cached = True
    bass2jax.compile_bir_kernel = cached


def _install_ntff_hook_shim():
    """Provide antenv.axon_hooks (absent in this container) so that
    run_bass_kernel_spmd(trace=True) can capture NTFF profiles via the
    axon .so — mirrors trn_agent_boot.trn_boot's ctypes hook."""
    import types
    import ctypes
    import contextlib

    if "antenv.axon_hooks" in sys.modules:
        return
    so_path = "/opt/axon/libaxon_pjrt.so"
    lib = ctypes.CDLL(so_path)
    if not hasattr(lib, "axon_start_nrt_profile"):
        return
    lib.axon_start_nrt_profile.argtypes = [
        ctypes.POINTER(ctypes.c_int64),
        ctypes.c_size_t,
    ]
    lib.axon_start_nrt_profile.restype = ctypes.c_int64
    lib.axon_stop_nrt_profile.argtypes = [ctypes.c_char_p]
    lib.axon_stop_nrt_profile.restype = ctypes.c_int64

    @contextlib.contextmanager
    def _hook(output_dir, device_ids):
        import jax

        jax.devices()
        if device_ids:
            ids = (ctypes.c_int64 * len(device_ids))(*device_ids)
            rc = lib.axon_start_nrt_profile(ids, len(device_ids))
        else:
            rc = lib.axon_start_nrt_profile(None, 0)
        if rc != 0:
            raise RuntimeError(f"axon_start_nrt_profile rc={rc}")
        try:
            yield
        finally:
            n = lib.axon_stop_nrt_profile(str(output_dir).encode())
            print(f"profile: {n} file(s) written to {output_dir}", file=sys.stderr)

    mod = types.ModuleType("antenv.axon_hooks")
    mod.get_axon_ntff_profile_hook = lambda: _hook
    mod.set_axon_ntff_profile_hook = lambda h: None
    sys.modules["antenv.axon_hooks"] = mod


def _assign_experts(n16, n8):
    """Assign experts to cores minimizing sum_j roundup8(max_c sorted16[c][j])
    + 0.5 * (same for fp8): LPT on weighted load, then swap refinement."""
    rng = np.random.default_rng(12345)
    wload = n16 + 0.5 * n8
    order = np.argsort(-wload, kind="stable")
    cores = [[] for _ in range(NCORES)]
    tot = np.zeros(NCORES)
    for e in order:
        c = int(np.argmin(tot))
        cores[c].append(int(e))
        tot[c] += wload[e]
    assign = np.zeros(E, dtype=np.int64)
    for c, ids in enumerate(cores):
        assign[ids] = c

    def cost(asg):
        m16 = np.zeros((NCORES, EPC))
        m8 = np.zeros((NCORES, EPC))
        for c in range(NCORES):
            ids = np.where(asg == c)[0]
            m16[c] = np.sort(n16[ids])[::-1]
            m8[c] = np.sort(n8[ids])[::-1]
        w16 = np.ceil(m16.max(0) / 8) * 8
        w8 = np.ceil(m8.max(0) / 8) * 8
        return w16.sum() + 0.5 * w8.sum()

    best = cost(assign)
    for _ in range(4000):
        e1, e2 = rng.integers(0, E, 2)
        c1, c2 = assign[e1], assign[e2]
        if c1 == c2:
            continue
        assign[e1], assign[e2] = c2, c1
        cnew = cost(assign)
        if cnew < best:
            best = cnew
        else:
            assign[e1], assign[e2] = c1, c2
    return assign


def kernel(
    hidden_states,
    routing_weights,
    selected_experts,
    gate_proj,
    up_proj,
    down_proj,
):
    global LAST_EXEC_NS, LAST_RESULTS
    from concourse.bass_utils import run_bass_kernel_spmd

    _install_neff_cache()

    x = np.ascontiguousarray(np.asarray(hidden_states, dtype=np.float32))
    rw = np.asarray(routing_weights, dtype=np.float32)
    sel = np.asarray(selected_experts).astype(np.int64)
    gate = np.asarray(gate_proj, dtype=np.float32)
    up = np.asarray(up_proj, dtype=np.float32)
    down = np.asarray(down_proj, dtype=np.float32)

    # ---- host dispatch (mirrors reference's stable sort-by-expert) ----
    flat_e = sel.reshape(-1)
    flat_p8 = rw.reshape(-1) < THETA  # fp8-path mask per (t, k) pair
    order = np.argsort(flat_e, kind="stable")
    sorted_e = flat_e[order]
    counts = np.bincount(flat_e, minlength=E)
    offsets = np.concatenate([[0], np.cumsum(counts)[:-1]])
    pos = np.arange(flat_e.shape[0], dtype=np.int64) - offsets[sorted_e]
    keep = pos < CAP  # reference capacity drop (pos within expert)

    p8_s = flat_p8[order]
    m16 = keep & ~p8_s
    m8 = keep & p8_s
    # rank within (expert, path), in stable order
    c16 = np.cumsum(m16)
    c8 = np.cumsum(m8)
    start16 = np.concatenate([[0], c16])[offsets[sorted_e]]
    start8 = np.concatenate([[0], c8])[offsets[sorted_e]]
    pos16 = c16 - 1 - start16  # valid where m16
    pos8 = c8 - 1 - start8
    n16 = np.bincount(sorted_e[m16], minlength=E)
    n8 = np.bincount(sorted_e[m8], minlength=E)

    # ---- expert -> core assignment + per-(path, slot) compile-time widths ----
    assign = _assign_experts(n16, n8)
    perm16 = np.zeros((NCORES, EPC), dtype=np.int64)
    perm8 = np.zeros((NCORES, EPC), dtype=np.int64)
    for c in range(NCORES):
        ids = np.where(assign == c)[0]
        perm16[c] = ids[np.argsort(-n16[ids], kind="stable")]
        perm8[c] = ids[np.argsort(-n8[ids], kind="stable")]
    w16s = tuple(
        int(min(CAP, max(16, -(-int(n16[perm16[:, j]].max()) // 8) * 8)))
        for j in range(EPC)
    )
    w8s = tuple(
        int(min(CAP, max(16, -(-int(n8[perm8[:, j]].max()) // 8) * 8)))
        for j in range(EPC)
    )

    tok = order // K
    maxw16, maxw8 = max(w16s), max(w8s)
    xbuf16 = np.zeros((E, maxw16, H), dtype=np.float32)
    xbuf16[sorted_e[m16], pos16[m16]] = x[tok[m16]]
    xbuf8 = np.zeros((E, maxw8, H), dtype=np.float32)
    xbuf8[sorted_e[m8], pos8[m8]] = x[tok[m8]]

    # ---- weight layouts (contiguous per-DMA blocks, shared fp16/fp8 shape) ----
    # gate/up slice (e, i): [128p, NH, 128c] where [p, h, c] = W[h*128+p, i*128+c]
    gate_r = gate.reshape(E, NH, 128, NI, 128).transpose(0, 3, 2, 1, 4)
    up_r = up.reshape(E, NH, 128, NI, 128).transpose(0, 3, 2, 1, 4)
    # down slice (e, h): [128p, NI, 128m] where [p, i, m] = W[i*128+p, h*128+m]
    down_r = down.reshape(E, NI, 128, NH, 128).transpose(0, 3, 2, 1, 4)
    gate16_full = np.ascontiguousarray(gate_r, dtype=np.float16)
    up16_full = np.ascontiguousarray(up_r, dtype=np.float16)
    down16_full = np.ascontiguousarray(down_r, dtype=np.float16)
    gate8_full = (gate_r * SG).astype(E4)
    up8_full = (up_r * SU).astype(E4)
    down8_full = (down_r * SD).astype(E4)

    key = (w16s, w8s)
    nc = _prog_cache.get(key)
    if nc is None:
        nc = _build_program(w16s, w8s)
        _prog_cache[key] = nc

    in_maps = []
    for c in range(NCORES):
        m = {
            "gw16": np.ascontiguousarray(gate16_full[perm16[c]]),
            "uw16": np.ascontiguousarray(up16_full[perm16[c]]),
            "dw16": np.ascontiguousarray(down16_full[perm16[c]]),
            "gw8": np.ascontiguousarray(gate8_full[perm8[c]]),
            "uw8": np.ascontiguousarray(up8_full[perm8[c]]),
            "dw8": np.ascontiguousarray(down8_full[perm8[c]]),
        }
        for j in range(EPC):
            e16, w16_ = perm16[c, j], w16s[j]
            e8, w8_ = perm8[c, j], w8s[j]
            m[f"x16_{j}"] = np.ascontiguousarray(
                xbuf16[e16, :w16_].T.reshape(NH, 128, w16_), dtype=np.float16
            )
            m[f"x8_{j}"] = np.ascontiguousarray(
                xbuf8[e8, :w8_].T.reshape(NH, 128, w8_)
            ).astype(E4)
        in_maps.append(m)

    trace = bool(os.environ.get("BASS_MOE_TRACE"))
    kwargs = {}
    if trace:
        _install_ntff_hook_shim()
        tcores = os.environ.get("BASS_MOE_TRACE_CORES", "0")
        kwargs = dict(trace=True, trace_cores=[int(c) for c in tcores.split(",")])
    res = run_bass_kernel_spmd(nc, in_maps, core_ids=list(range(NCORES)), **kwargs)
    LAST_EXEC_NS = res.exec_time_ns
    LAST_RESULTS = res

    # ---- host combine ----
    o16 = np.zeros((E, maxw16, H), dtype=np.float32)
    o8 = np.zeros((E, maxw8, H), dtype=np.float32)
    for c in range(NCORES):
        for j in range(EPC):
            # y arrays are [NH//4, 128p, 4hq, w]: (q, p, hq, n) = O^T[(4q+hq)*128+p, n]
            w = w16s[j]
            o16[perm16[c, j], :w] = (
                res.results[c][f"y16_{j}"]
                .astype(np.float32)
                .transpose(0, 2, 1, 3)
                .reshape(H, w)
                .T
            )
            w = w8s[j]
            o8[perm8[c, j], :w] = (
                res.results[c][f"y8_{j}"]
                .astype(np.float32)
                .transpose(0, 2, 1, 3)
                .reshape(H, w)
                .T
            )
    o8 *= 1.0 / (SU * SD)

    gathered = np.zeros((flat_e.shape[0], H), dtype=np.float32)
    gathered[order[m16]] = o16[sorted_e[m16], pos16[m16]]
    gathered[order[m8]] = o8[sorted_e[m8], pos8[m8]]
    y = (gathered.reshape(T, K, H) * rw[:, :, None]).sum(axis=1, dtype=np.float32)
    return y.astype(np.float32)


# revision 33
# speedup vs baseline: 1.1003x; 1.1003x over previous
"""MoE experts kernel for Trainium2 (8 NeuronCores, expert-parallel),
mixed-precision: per (token, expert) pair, low routing weight -> fp8
(e4m3 DoubleRow matmuls, 2x PE rate), high routing weight -> fp16.

Reference computation (per token t, top-k expert e with gate p):
    y[t] = sum_k p[t,k] * down_e @ (silu(x[t] @ gate_e) * (x[t] @ up_e))
with per-expert capacity CAP=1024 (tokens beyond capacity dropped).

Error budget: final tolerance 2e-2. fp8-everything measures 6.6e-2;
routing pairs with p < THETA=0.09 (~35% of pairs, ~4.7% of sum p^2)
through the fp8 path gives ~1.4e-2 end-to-end, while cutting PE work
by ~17% (fp8 DoubleRow contracts 256 rows/instruction at the same
per-column rate as fp16's 128).

Layout trick: a [128, NH, 128] stationary weight tile serves both
paths -- fp16 matmuls slice [:, h, :], fp8 DoubleRow slices
[:, 2hh:2hh+2, :] (the pair dim is just two adjacent h-chunks).

Scales (fp8 path): gate*64 (undone by silu's input scale), up*4,
down*16 -> device output = 64*o, fp16-safe; host divides by 64.
"""

import os
import sys

sys.path.insert(0, "/opt/trn_rl_repo")

import numpy as np
import ml_dtypes

E, H, I, T, K = 64, 2048, 768, 4096, 8
CAP = 1024
NCORES = 8
EPC = E // NCORES  # experts per core
NH = H // 128  # 16 contraction chunks for gate/up
NI = I // 128  # 6 contraction chunks for down

THETA = 0.10  # routing-weight threshold: p < THETA -> fp8 path
SG, SU, SD = 64.0, 4.0, 16.0  # fp8 quantization scales
E4 = ml_dtypes.float8_e4m3

_prog_cache = {}
LAST_EXEC_NS = None
LAST_RESULTS = None


def _groups(npad):
    ng = -(-npad // 512)
    w = -(-npad // ng)
    out = []
    s = 0
    while s < npad:
        e = min(s + w, npad)
        out.append((s, e))
        s = e
    return out


def _build_program(w16s, w8s):
    import concourse.bacc as bacc
    import concourse.mybir as mybir
    from concourse.tile import TileContext

    f32 = mybir.dt.float32
    f16 = mybir.dt.float16
    f8 = mybir.dt.float8e4
    DR = mybir.MatmulPerfMode.DoubleRow
    SILU = mybir.ActivationFunctionType.Silu

    nc = bacc.Bacc(None, target_bir_lowering=False)
    xT16s = [
        nc.declare_dram_parameter(f"x16_{j}", [NH, 128, w], f16, isOutput=False)
        for j, w in enumerate(w16s)
    ]
    xT8s = [
        nc.declare_dram_parameter(f"x8_{j}", [NH, 128, w], f8, isOutput=False)
        for j, w in enumerate(w8s)
    ]
    gw16 = nc.declare_dram_parameter("gw16", [EPC, NI, 128, NH, 128], f16, isOutput=False)
    uw16 = nc.declare_dram_parameter("uw16", [EPC, NI, 128, NH, 128], f16, isOutput=False)
    dw16 = nc.declare_dram_parameter("dw16", [EPC, NH, 128, NI, 128], f16, isOutput=False)
    gw8 = nc.declare_dram_parameter("gw8", [EPC, NI, 128, NH, 128], f8, isOutput=False)
    uw8 = nc.declare_dram_parameter("uw8", [EPC, NI, 128, NH, 128], f8, isOutput=False)
    dw8 = nc.declare_dram_parameter("dw8", [EPC, NH, 128, NI, 128], f8, isOutput=False)
    # [q, 128p, 4hq, w]: element (q, p, hq, n) = O^T[(4q+hq)*128+p... wait see
    # host combine — stored so the device DMA is layout-matched to the ot tile.
    yT16s = [
        nc.declare_dram_parameter(f"y16_{j}", [NH // 4, 128, 4, w], f16, isOutput=True)
        for j, w in enumerate(w16s)
    ]
    yT8s = [
        nc.declare_dram_parameter(f"y8_{j}", [NH // 4, 128, 4, w], f16, isOutput=True)
        for j, w in enumerate(w8s)
    ]

    with TileContext(nc) as tc:
        with (
            tc.sbuf_pool(name="xp", bufs=2) as xp,
            tc.sbuf_pool(name="wp", bufs=3) as wp,
            tc.sbuf_pool(name="hp", bufs=2) as hp,
            tc.sbuf_pool(name="op", bufs=3) as op,
            tc.sbuf_pool(name="tp", bufs=3) as tp,
            tc.psum_pool(name="pp", bufs=2) as pp,
        ):
            for k in range(EPC):
                wa, wb = w16s[k], w8s[k]
                ga16, ga8 = _groups(wa), _groups(wb)
                # Three parallel DMA flows: gate/up weight stream alone on the
                # Sync DGE queue (the big stream, never blocked); down weights
                # early on the Scalar DGE queue (transfer during phase A);
                # x tiles and outputs on the idle GpSimd SWDGE queue.
                # expert 0's x tiles go on the fast Sync HW queue, interleaved
                # with the first gate/up weights so the PE starts ASAP (SWDGE
                # takes ~5us/DMA to spin up descriptor gen — fine once
                # prefetch is a full expert ahead, fatal on startup)
                xr16 = xT16s[k].rearrange("h p n -> p h n")
                xr8 = xT8s[k].rearrange("h p n -> p h n")
                xts16 = []
                xts8 = []
                pre = {}
                if k == 0:
                    # startup fast path, all on the Sync HW queue in the exact
                    # order the PE consumes: first x chunk, first gate/up
                    # weights, remaining x chunks, first fp8 weights. (SWDGE
                    # takes ~5us/DMA to spin up; HW DGE delivers immediately.)
                    xt = xp.tile([128, 4, wa], f16, name="xt16_0", tag="xt16_0")
                    nc.sync.dma_start(out=xt, in_=xr16[:, 0:4, :])
                    xts16.append(xt)
                    g16 = wp.tile([128, NH, 128], f16, name="g16", tag="g16", bufs=3)
                    u16 = wp.tile([128, NH, 128], f16, name="u16", tag="u16", bufs=3)
                    nc.sync.dma_start(out=g16, in_=gw16[k, 0, :, :, :])
                    nc.sync.dma_start(out=u16, in_=uw16[k, 0, :, :, :])
                    for j in range(1, 4):
                        xt = xp.tile([128, 4, wa], f16, name=f"xt16_{j}", tag=f"xt16_{j}")
                        nc.sync.dma_start(out=xt, in_=xr16[:, 4 * j : 4 * (j + 1), :])
                        xts16.append(xt)
                    for j in range(4):
                        xt = xp.tile([128, 4, wb], f8, name=f"xt8_{j}", tag=f"xt8_{j}")
                        nc.sync.dma_start(out=xt, in_=xr8[:, 4 * j : 4 * (j + 1), :])
                        xts8.append(xt)
                    g8 = wp.tile([128, NH, 128], f8, name="g8", tag="g8")
                    u8 = wp.tile([128, NH, 128], f8, name="u8", tag="u8")
                    nc.sync.dma_start(out=g8, in_=gw8[k, 0, :, :, :])
                    nc.sync.dma_start(out=u8, in_=uw8[k, 0, :, :, :])
                    pre[0] = (g16, u16, g8, u8)
                else:
                    for j in range(4):
                        xt = xp.tile([128, 4, wa], f16, name=f"xt16_{j}", tag=f"xt16_{j}")
                        nc.gpsimd.dma_start(out=xt, in_=xr16[:, 4 * j : 4 * (j + 1), :])
                        xts16.append(xt)
                    for j in range(4):
                        xt = xp.tile([128, 4, wb], f8, name=f"xt8_{j}", tag=f"xt8_{j}")
                        nc.gpsimd.dma_start(out=xt, in_=xr8[:, 4 * j : 4 * (j + 1), :])
                        xts8.append(xt)
                d16 = wp.tile([128, NH, NI, 128], f16, name="d16", tag="d16", bufs=2)
                d8 = wp.tile([128, NH, NI, 128], f8, name="d8", tag="d8", bufs=2)
                dr16 = dw16[k].rearrange("h p i m -> p h i m")
                dr8 = dw8[k].rearrange("h p i m -> p h i m")
                if k > 0:
                    # k=0's down-weight DMAs are deferred into the i-loop so
                    # the first silu groups aren't stuck behind them on the
                    # Scalar queue
                    nc.scalar.dma_start(out=d16[:, 0 : NH // 2, :, :], in_=dr16[:, 0 : NH // 2, :, :])
                    nc.scalar.dma_start(out=d16[:, NH // 2 :, :, :], in_=dr16[:, NH // 2 :, :, :])
                    nc.scalar.dma_start(out=d8[:, 0 : NH // 2, :, :], in_=dr8[:, 0 : NH // 2, :, :])
                    nc.scalar.dma_start(out=d8[:, NH // 2 :, :, :], in_=dr8[:, NH // 2 :, :, :])
                hm16 = hp.tile([128, NI, wa], f16, name="hm16", tag="hm16")
                hm8 = hp.tile([128, NI, wb], f8, name="hm8", tag="hm8")
                for i in range(NI):
                    if k == 0 and i == 1:
                        nc.scalar.dma_start(out=d16[:, 0 : NH // 2, :, :], in_=dr16[:, 0 : NH // 2, :, :])
                        nc.scalar.dma_start(out=d16[:, NH // 2 :, :, :], in_=dr16[:, NH // 2 :, :, :])
                        nc.scalar.dma_start(out=d8[:, 0 : NH // 2, :, :], in_=dr8[:, 0 : NH // 2, :, :])
                        nc.scalar.dma_start(out=d8[:, NH // 2 :, :, :], in_=dr8[:, NH // 2 :, :, :])
                    if i in pre:
                        g16, u16, g8, u8 = pre[i]
                    else:
                        g16 = wp.tile([128, NH, 128], f16, name="g16", tag="g16", bufs=3)
                        u16 = wp.tile([128, NH, 128], f16, name="u16", tag="u16", bufs=3)
                        nc.sync.dma_start(out=g16, in_=gw16[k, i, :, :, :])
                        nc.sync.dma_start(out=u16, in_=uw16[k, i, :, :, :])
                        g8 = wp.tile([128, NH, 128], f8, name="g8", tag="g8")
                        u8 = wp.tile([128, NH, 128], f8, name="u8", tag="u8")
                        nc.sync.dma_start(out=g8, in_=gw8[k, i, :, :, :])
                        nc.sync.dma_start(out=u8, in_=uw8[k, i, :, :, :])
                    for g0, g1 in ga16:
                        wdt = g1 - g0
                        psg = pp.tile([128, wdt], f32, name="psg", tag="psg", bufs=3)
                        psu = pp.tile([128, wdt], f32, name="psu", tag="psu", bufs=2)
                        for h in range(NH):
                            nc.tensor.matmul(
                                psg, g16[:, h, :], xts16[h // 4][:, h % 4, g0:g1],
                                start=(h == 0), stop=(h == NH - 1),
                            )
                        for h in range(NH):
                            nc.tensor.matmul(
                                psu, u16[:, h, :], xts16[h // 4][:, h % 4, g0:g1],
                                start=(h == 0), stop=(h == NH - 1),
                            )
                        sil = tp.tile([128, wdt], f32, name="sil", tag="sil")
                        nc.scalar.activation(sil, psg, SILU)
                        nc.vector.tensor_mul(hm16[:, i, g0:g1], sil, psu)
                    for g0, g1 in ga8:
                        wdt = g1 - g0
                        psg8 = pp.tile([128, wdt], f32, name="psg8", tag="psg", bufs=3)
                        psu8 = pp.tile([128, wdt], f32, name="psu8", tag="psu", bufs=2)
                        for hh in range(NH // 2):
                            m = hh % 2
                            nc.tensor.matmul(
                                psg8,
                                g8[:, 2 * hh : 2 * hh + 2, :],
                                xts8[hh // 2][:, 2 * m : 2 * m + 2, g0:g1],
                                start=(hh == 0), stop=(hh == NH // 2 - 1),
                                perf_mode=DR,
                            )
                        for hh in range(NH // 2):
                            m = hh % 2
                            nc.tensor.matmul(
                                psu8,
                                u8[:, 2 * hh : 2 * hh + 2, :],
                                xts8[hh // 2][:, 2 * m : 2 * m + 2, g0:g1],
                                start=(hh == 0), stop=(hh == NH // 2 - 1),
                                perf_mode=DR,
                            )
                        sil8 = tp.tile([128, wdt], f32, name="sil8", tag="sil")
                        nc.scalar.activation(sil8, psg8, SILU, scale=1.0 / SG)
                        nc.vector.tensor_mul(hm8[:, i, g0:g1], sil8, psu8)
                for q in range(NH // 4):
                    ot16 = op.tile([128, 4, wa], f16, name="ot16", tag="ot16")
                    ot8 = op.tile([128, 4, wb], f16, name="ot8", tag="ot8")
                    for hq in range(4):
                        h = 4 * q + hq
                        for g0, g1 in ga16:
                            wdt = g1 - g0
                            pso = pp.tile([128, wdt], f32, name="pso", tag="pso", bufs=3)
                            for i in range(NI):
                                nc.tensor.matmul(
                                    pso, d16[:, h, i, :], hm16[:, i, g0:g1],
                                    start=(i == 0), stop=(i == NI - 1),
                                )
                            nc.vector.tensor_copy(ot16[:, hq, g0:g1], pso)
                        for g0, g1 in ga8:
                            wdt = g1 - g0
                            pso8 = pp.tile([128, wdt], f32, name="pso8", tag="pso", bufs=3)
                            for ii in range(NI // 2):
                                nc.tensor.matmul(
                                    pso8,
                                    d8[:, h, 2 * ii : 2 * ii + 2, :],
                                    hm8[:, 2 * ii : 2 * ii + 2, g0:g1],
                                    start=(ii == 0), stop=(ii == NI // 2 - 1),
                                    perf_mode=DR,
                                )
                            nc.scalar.activation(
                                ot8[:, hq, g0:g1], pso8,
                                mybir.ActivationFunctionType.Identity,
                            )
                    nc.gpsimd.dma_start(out=yT16s[k][q, :, :, :], in_=ot16)
                    nc.gpsimd.dma_start(out=yT8s[k][q, :, :, :], in_=ot8)
    nc.compile()
    return nc


def _install_neff_cache():
    """Cache walrus NEFF compiles on disk keyed by BIR hash (compile of the
    ~10k-instruction program takes minutes; the BIR is deterministic)."""
    import hashlib
    import shutil

    import concourse.bass2jax as bass2jax
    from concourse.bass_utils import compile_bir_kernel as _orig

    if getattr(bass2jax.compile_bir_kernel, "_moe_cached", False):
        return
    cache_dir = os.environ.get("BASS_NEFF_CACHE", "/tmp/bass_neff_cache")
    try:
        os.makedirs(cache_dir, exist_ok=True)
    except OSError:
        return

    def cached(bir_json, tmpdir, neff_name="file.neff"):
        key = hashlib.sha256(bir_json).hexdigest()[:24]
        cpath = os.path.join(cache_dir, key + ".neff")
        dst = os.path.join(tmpdir, neff_name)
        if os.path.exists(cpath):
            shutil.copy(cpath, dst)
            return dst
        out = _orig(bir_json, tmpdir, neff_name)
        try:
            shutil.copy(out, cpath)
        except OSError:
            pass
        return out

    cached._moe_cached = True
    bass2jax.compile_bir_kernel = cached


def _install_ntff_hook_shim():
    """Provide antenv.axon_hooks (absent in this container) so that
    run_bass_kernel_spmd(trace=True) can capture NTFF profiles via the
    axon .so — mirrors trn_agent_boot.trn_boot's ctypes hook."""
    import types
    import ctypes
    import contextlib

    if "antenv.axon_hooks" in sys.modules:
        return
    so_path = "/opt/axon/libaxon_pjrt.so"
    lib = ctypes.CDLL(so_path)
    if not hasattr(lib, "axon_start_nrt_profile"):
        return
    lib.axon_start_nrt_profile.argtypes = [
        ctypes.POINTER(ctypes.c_int64),
        ctypes.c_size_t,
    ]
    lib.axon_start_nrt_profile.restype = ctypes.c_int64
    lib.axon_stop_nrt_profile.argtypes = [ctypes.c_char_p]
    lib.axon_stop_nrt_profile.restype = ctypes.c_int64

    @contextlib.contextmanager
    def _hook(output_dir, device_ids):
        import jax

        jax.devices()
        if device_ids:
            ids = (ctypes.c_int64 * len(device_ids))(*device_ids)
            rc = lib.axon_start_nrt_profile(ids, len(device_ids))
        else:
            rc = lib.axon_start_nrt_profile(None, 0)
        if rc != 0:
            raise RuntimeError(f"axon_start_nrt_profile rc={rc}")
        try:
            yield
        finally:
            n = lib.axon_stop_nrt_profile(str(output_dir).encode())
            print(f"profile: {n} file(s) written to {output_dir}", file=sys.stderr)

    mod = types.ModuleType("antenv.axon_hooks")
    mod.get_axon_ntff_profile_hook = lambda: _hook
    mod.set_axon_ntff_profile_hook = lambda h: None
    sys.modules["antenv.axon_hooks"] = mod


def _assign_experts(n16, n8):
    """Assign experts to cores minimizing sum_j roundup8(max_c sorted16[c][j])
    + 0.5 * (same for fp8): LPT on weighted load, then swap refinement."""
    rng = np.random.default_rng(12345)
    wload = n16 + 0.5 * n8
    order = np.argsort(-wload, kind="stable")
    cores = [[] for _ in range(NCORES)]
    tot = np.zeros(NCORES)
    for e in order:
        c = int(np.argmin(tot))
        cores[c].append(int(e))
        tot[c] += wload[e]
    assign = np.zeros(E, dtype=np.int64)
    for c, ids in enumerate(cores):
        assign[ids] = c

    def cost(asg):
        m16 = np.zeros((NCORES, EPC))
        m8 = np.zeros((NCORES, EPC))
        for c in range(NCORES):
            ids = np.where(asg == c)[0]
            m16[c] = np.sort(n16[ids])[::-1]
            m8[c] = np.sort(n8[ids])[::-1]
        w16 = np.ceil(m16.max(0) / 8) * 8
        w8 = np.ceil(m8.max(0) / 8) * 8
        return w16.sum() + 0.5 * w8.sum()

    best = cost(assign)
    for _ in range(4000):
        e1, e2 = rng.integers(0, E, 2)
        c1, c2 = assign[e1], assign[e2]
        if c1 == c2:
            continue
        assign[e1], assign[e2] = c2, c1
        cnew = cost(assign)
        if cnew < best:
            best = cnew
        else:
            assign[e1], assign[e2] = c1, c2
    return assign


def kernel(
    hidden_states,
    routing_weights,
    selected_experts,
    gate_proj,
    up_proj,
    down_proj,
):
    global LAST_EXEC_NS, LAST_RESULTS
    from concourse.bass_utils import run_bass_kernel_spmd

    _install_neff_cache()

    x = np.ascontiguousarray(np.asarray(hidden_states, dtype=np.float32))
    rw = np.asarray(routing_weights, dtype=np.float32)
    sel = np.asarray(selected_experts).astype(np.int64)
    gate = np.asarray(gate_proj, dtype=np.float32)
    up = np.asarray(up_proj, dtype=np.float32)
    down = np.asarray(down_proj, dtype=np.float32)

    # ---- host dispatch (mirrors reference's stable sort-by-expert) ----
    flat_e = sel.reshape(-1)
    flat_p8 = rw.reshape(-1) < THETA  # fp8-path mask per (t, k) pair
    order = np.argsort(flat_e, kind="stable")
    sorted_e = flat_e[order]
    counts = np.bincount(flat_e, minlength=E)
    offsets = np.concatenate([[0], np.cumsum(counts)[:-1]])
    pos = np.arange(flat_e.shape[0], dtype=np.int64) - offsets[sorted_e]
    keep = pos < CAP  # reference capacity drop (pos within expert)

    p8_s = flat_p8[order]
    m16 = keep & ~p8_s
    m8 = keep & p8_s
    # rank within (expert, path), in stable order
    c16 = np.cumsum(m16)
    c8 = np.cumsum(m8)
    start16 = np.concatenate([[0], c16])[offsets[sorted_e]]
    start8 = np.concatenate([[0], c8])[offsets[sorted_e]]
    pos16 = c16 - 1 - start16  # valid where m16
    pos8 = c8 - 1 - start8
    n16 = np.bincount(sorted_e[m16], minlength=E)
    n8 = np.bincount(sorted_e[m8], minlength=E)

    # ---- expert -> core assignment + per-(path, slot) compile-time widths ----
    assign = _assign_experts(n16, n8)
    perm16 = np.zeros((NCORES, EPC), dtype=np.int64)
    perm8 = np.zeros((NCORES, EPC), dtype=np.int64)
    for c in range(NCORES):
        ids = np.where(assign == c)[0]
        perm16[c] = ids[np.argsort(-n16[ids], kind="stable")]
        perm8[c] = ids[np.argsort(-n8[ids], kind="stable")]
    w16s = tuple(
        int(min(CAP, max(16, -(-int(n16[perm16[:, j]].max()) // 8) * 8)))
        for j in range(EPC)
    )
    w8s = tuple(
        int(min(CAP, max(16, -(-int(n8[perm8[:, j]].max()) // 8) * 8)))
        for j in range(EPC)
    )

    tok = order // K
    maxw16, maxw8 = max(w16s), max(w8s)
    xbuf16 = np.zeros((E, maxw16, H), dtype=np.float32)
    xbuf16[sorted_e[m16], pos16[m16]] = x[tok[m16]]
    xbuf8 = np.zeros((E, maxw8, H), dtype=np.float32)
    xbuf8[sorted_e[m8], pos8[m8]] = x[tok[m8]]

    # ---- weight layouts (contiguous per-DMA blocks, shared fp16/fp8 shape) ----
    # gate/up slice (e, i): [128p, NH, 128c] where [p, h, c] = W[h*128+p, i*128+c]
    gate_r = gate.reshape(E, NH, 128, NI, 128).transpose(0, 3, 2, 1, 4)
    up_r = up.reshape(E, NH, 128, NI, 128).transpose(0, 3, 2, 1, 4)
    # down slice (e, h): [128p, NI, 128m] where [p, i, m] = W[i*128+p, h*128+m]
    down_r = down.reshape(E, NI, 128, NH, 128).transpose(0, 3, 2, 1, 4)
    gate16_full = np.ascontiguousarray(gate_r, dtype=np.float16)
    up16_full = np.ascontiguousarray(up_r, dtype=np.float16)
    down16_full = np.ascontiguousarray(down_r, dtype=np.float16)
    gate8_full = (gate_r * SG).astype(E4)
    up8_full = (up_r * SU).astype(E4)
    down8_full = (down_r * SD).astype(E4)

    key = (w16s, w8s)
    nc = _prog_cache.get(key)
    if nc is None:
        nc = _build_program(w16s, w8s)
        _prog_cache[key] = nc

    in_maps = []
    for c in range(NCORES):
        m = {
            "gw16": np.ascontiguousarray(gate16_full[perm16[c]]),
            "uw16": np.ascontiguousarray(up16_full[perm16[c]]),
            "dw16": np.ascontiguousarray(down16_full[perm16[c]]),
            "gw8": np.ascontiguousarray(gate8_full[perm8[c]]),
            "uw8": np.ascontiguousarray(up8_full[perm8[c]]),
            "dw8": np.ascontiguousarray(down8_full[perm8[c]]),
        }
        for j in range(EPC):
            e16, w16_ = perm16[c, j], w16s[j]
            e8, w8_ = perm8[c, j], w8s[j]
            m[f"x16_{j}"] = np.ascontiguousarray(
                xbuf16[e16, :w16_].T.reshape(NH, 128, w16_), dtype=np.float16
            )
            m[f"x8_{j}"] = np.ascontiguousarray(
                xbuf8[e8, :w8_].T.reshape(NH, 128, w8_)
            ).astype(E4)
        in_maps.append(m)

    trace = bool(os.environ.get("BASS_MOE_TRACE"))
    kwargs = {}
    if trace:
        _install_ntff_hook_shim()
        tcores = os.environ.get("BASS_MOE_TRACE_CORES", "0")
        kwargs = dict(trace=True, trace_cores=[int(c) for c in tcores.split(",")])
    res = run_bass_kernel_spmd(nc, in_maps, core_ids=list(range(NCORES)), **kwargs)
    LAST_EXEC_NS = res.exec_time_ns
    LAST_RESULTS = res

    # ---- host combine ----
    o16 = np.zeros((E, maxw16, H), dtype=np.float32)
    o8 = np.zeros((E, maxw8, H), dtype=np.float32)
    for c in range(NCORES):
        for j in range(EPC):
            # y arrays are [NH//4, 128p, 4hq, w]: (q, p, hq, n) = O^T[(4q+hq)*128+p, n]
            w = w16s[j]
            o16[perm16[c, j], :w] = (
                res.results[c][f"y16_{j}"]
                .astype(np.float32)
                .transpose(0, 2, 1, 3)
                .reshape(H, w)
                .T
            )
            w = w8s[j]
            o8[perm8[c, j], :w] = (
                res.results[c][f"y8_{j}"]
                .astype(np.float32)
                .transpose(0, 2, 1, 3)
                .reshape(H, w)
                .T
            )
    o8 *= 1.0 / (SU * SD)

    gathered = np.zeros((flat_e.shape[0], H), dtype=np.float32)
    gathered[order[m16]] = o16[sorted_e[m16], pos16[m16]]
    gathered[order[m8]] = o8[sorted_e[m8], pos8[m8]]
    y = (gathered.reshape(T, K, H) * rw[:, :, None]).sum(axis=1, dtype=np.float32)
    return y.astype(np.float32)


# revision 34
# speedup vs baseline: 1.1497x; 1.0449x over previous
"""MoE experts kernel for Trainium2 (8 NeuronCores, expert-parallel),
mixed-precision: per (token, expert) pair, low routing weight -> fp8
(e4m3 DoubleRow matmuls, 2x PE rate), high routing weight -> fp16.

Reference computation (per token t, top-k expert e with gate p):
    y[t] = sum_k p[t,k] * down_e @ (silu(x[t] @ gate_e) * (x[t] @ up_e))
with per-expert capacity CAP=1024 (tokens beyond capacity dropped).

Error budget: final tolerance 2e-2. fp8-everything measures 6.6e-2;
routing pairs with p < THETA=0.09 (~35% of pairs, ~4.7% of sum p^2)
through the fp8 path gives ~1.4e-2 end-to-end, while cutting PE work
by ~17% (fp8 DoubleRow contracts 256 rows/instruction at the same
per-column rate as fp16's 128).

Layout trick: a [128, NH, 128] stationary weight tile serves both
paths -- fp16 matmuls slice [:, h, :], fp8 DoubleRow slices
[:, 2hh:2hh+2, :] (the pair dim is just two adjacent h-chunks).

Scales (fp8 path): gate*64 (undone by silu's input scale), up*4,
down*16 -> device output = 64*o, fp16-safe; host divides by 64.
"""

import os
import sys

sys.path.insert(0, "/opt/trn_rl_repo")

import numpy as np
import ml_dtypes

E, H, I, T, K = 64, 2048, 768, 4096, 8
CAP = 1024
NCORES = 8
EPC = E // NCORES  # experts per core
NH = H // 128  # 16 contraction chunks for gate/up
NI = I // 128  # 6 contraction chunks for down

THETA = 0.10  # routing-weight threshold: p < THETA -> fp8 path
SG, SU, SD = 64.0, 4.0, 16.0  # fp8 quantization scales
E4 = ml_dtypes.float8_e4m3

_prog_cache = {}
LAST_EXEC_NS = None
LAST_RESULTS = None


def _groups(npad):
    ng = -(-npad // 512)
    w = -(-npad // ng)
    out = []
    s = 0
    while s < npad:
        e = min(s + w, npad)
        out.append((s, e))
        s = e
    return out


def _build_program(w16s, w8s):
    import concourse.bacc as bacc
    import concourse.mybir as mybir
    from concourse.tile import TileContext

    f32 = mybir.dt.float32
    f16 = mybir.dt.float16
    f8 = mybir.dt.float8e4
    DR = mybir.MatmulPerfMode.DoubleRow
    SILU = mybir.ActivationFunctionType.Silu

    nc = bacc.Bacc(None, target_bir_lowering=False)
    xT16s = [
        nc.declare_dram_parameter(f"x16_{j}", [NH, 128, w], f16, isOutput=False)
        for j, w in enumerate(w16s)
    ]
    xT8s = [
        nc.declare_dram_parameter(f"x8_{j}", [NH, 128, w], f8, isOutput=False)
        for j, w in enumerate(w8s)
    ]
    gw16 = nc.declare_dram_parameter("gw16", [EPC, NI, 128, NH, 128], f16, isOutput=False)
    uw16 = nc.declare_dram_parameter("uw16", [EPC, NI, 128, NH, 128], f16, isOutput=False)
    dw16 = nc.declare_dram_parameter("dw16", [EPC, NH, 128, NI, 128], f16, isOutput=False)
    gw8 = nc.declare_dram_parameter("gw8", [EPC, NI, 128, NH, 128], f8, isOutput=False)
    uw8 = nc.declare_dram_parameter("uw8", [EPC, NI, 128, NH, 128], f8, isOutput=False)
    dw8 = nc.declare_dram_parameter("dw8", [EPC, NH, 128, NI, 128], f8, isOutput=False)
    # [q, 128p, 4hq, w]: element (q, p, hq, n) = O^T[(4q+hq)*128+p... wait see
    # host combine — stored so the device DMA is layout-matched to the ot tile.
    yT16s = [
        nc.declare_dram_parameter(f"y16_{j}", [NH // 4, 128, 4, w], f16, isOutput=True)
        for j, w in enumerate(w16s)
    ]
    yT8s = [
        nc.declare_dram_parameter(f"y8_{j}", [NH // 4, 128, 4, w], f16, isOutput=True)
        for j, w in enumerate(w8s)
    ]

    with TileContext(nc) as tc:
        with (
            tc.sbuf_pool(name="xp", bufs=2) as xp,
            tc.sbuf_pool(name="wp", bufs=3) as wp,
            tc.sbuf_pool(name="hp", bufs=2) as hp,
            tc.sbuf_pool(name="op", bufs=3) as op,
            tc.sbuf_pool(name="tp", bufs=3) as tp,
            tc.psum_pool(name="pp", bufs=2) as pp,
        ):
            for k in range(EPC):
                wa, wb = w16s[k], w8s[k]
                ga16, ga8 = _groups(wa), _groups(wb)
                # Three parallel DMA flows: gate/up weight stream alone on the
                # Sync DGE queue (the big stream, never blocked); down weights
                # early on the Scalar DGE queue (transfer during phase A);
                # x tiles and outputs on the idle GpSimd SWDGE queue.
                # expert 0's x tiles go on the fast Sync HW queue, interleaved
                # with the first gate/up weights so the PE starts ASAP (SWDGE
                # takes ~5us/DMA to spin up descriptor gen — fine once
                # prefetch is a full expert ahead, fatal on startup)
                xr16 = xT16s[k].rearrange("h p n -> p h n")
                xr8 = xT8s[k].rearrange("h p n -> p h n")
                xts16 = []
                xts8 = []
                pre = {}
                xt = xp.tile([128, 4, wa], f16, name="xt16_0", tag="xt16_0")
                nc.sync.dma_start(out=xt, in_=xr16[:, 0:4, :])
                xts16.append(xt)
                if k == 0:
                    # startup: first gate/up weights right behind the first x
                    # chunk, in PE consumption order
                    g16 = wp.tile([128, NH, 128], f16, name="g16", tag="g16", bufs=3)
                    u16 = wp.tile([128, NH, 128], f16, name="u16", tag="u16", bufs=3)
                    nc.sync.dma_start(out=g16, in_=gw16[k, 0, :, :, :])
                    nc.sync.dma_start(out=u16, in_=uw16[k, 0, :, :, :])
                    g8 = wp.tile([128, NH, 128], f8, name="g8", tag="g8")
                    u8 = wp.tile([128, NH, 128], f8, name="u8", tag="u8")
                    nc.sync.dma_start(out=g8, in_=gw8[k, 0, :, :, :])
                    nc.sync.dma_start(out=u8, in_=uw8[k, 0, :, :, :])
                    pre[0] = (g16, u16, g8, u8)
                for j in range(1, 4):
                    xt = xp.tile([128, 4, wa], f16, name=f"xt16_{j}", tag=f"xt16_{j}")
                    nc.sync.dma_start(out=xt, in_=xr16[:, 4 * j : 4 * (j + 1), :])
                    xts16.append(xt)
                for j in range(4):
                    xt = xp.tile([128, 4, wb], f8, name=f"xt8_{j}", tag=f"xt8_{j}")
                    nc.sync.dma_start(out=xt, in_=xr8[:, 4 * j : 4 * (j + 1), :])
                    xts8.append(xt)
                d16 = wp.tile([128, NH, NI, 128], f16, name="d16", tag="d16", bufs=2)
                d8 = wp.tile([128, NH, NI, 128], f8, name="d8", tag="d8", bufs=2)
                dr16 = dw16[k].rearrange("h p i m -> p h i m")
                dr8 = dw8[k].rearrange("h p i m -> p h i m")
                if k > 0:
                    # k=0's down-weight DMAs are deferred into the i-loop so
                    # the first silu groups aren't stuck behind them on the
                    # Scalar queue
                    nc.scalar.dma_start(out=d16[:, 0 : NH // 2, :, :], in_=dr16[:, 0 : NH // 2, :, :])
                    nc.scalar.dma_start(out=d16[:, NH // 2 :, :, :], in_=dr16[:, NH // 2 :, :, :])
                    nc.scalar.dma_start(out=d8[:, 0 : NH // 2, :, :], in_=dr8[:, 0 : NH // 2, :, :])
                    nc.scalar.dma_start(out=d8[:, NH // 2 :, :, :], in_=dr8[:, NH // 2 :, :, :])
                hm16 = hp.tile([128, NI, wa], f16, name="hm16", tag="hm16")
                hm8 = hp.tile([128, NI, wb], f8, name="hm8", tag="hm8")
                for i in range(NI):
                    if k == 0 and i == 1:
                        nc.scalar.dma_start(out=d16[:, 0 : NH // 2, :, :], in_=dr16[:, 0 : NH // 2, :, :])
                        nc.scalar.dma_start(out=d16[:, NH // 2 :, :, :], in_=dr16[:, NH // 2 :, :, :])
                        nc.scalar.dma_start(out=d8[:, 0 : NH // 2, :, :], in_=dr8[:, 0 : NH // 2, :, :])
                        nc.scalar.dma_start(out=d8[:, NH // 2 :, :, :], in_=dr8[:, NH // 2 :, :, :])
                    if i in pre:
                        g16, u16, g8, u8 = pre[i]
                    else:
                        g16 = wp.tile([128, NH, 128], f16, name="g16", tag="g16", bufs=3)
                        u16 = wp.tile([128, NH, 128], f16, name="u16", tag="u16", bufs=3)
                        nc.sync.dma_start(out=g16, in_=gw16[k, i, :, :, :])
                        nc.sync.dma_start(out=u16, in_=uw16[k, i, :, :, :])
                        g8 = wp.tile([128, NH, 128], f8, name="g8", tag="g8")
                        u8 = wp.tile([128, NH, 128], f8, name="u8", tag="u8")
                        nc.sync.dma_start(out=g8, in_=gw8[k, i, :, :, :])
                        nc.sync.dma_start(out=u8, in_=uw8[k, i, :, :, :])
                    for g0, g1 in ga16:
                        wdt = g1 - g0
                        psg = pp.tile([128, wdt], f32, name="psg", tag="psg", bufs=3)
                        psu = pp.tile([128, wdt], f32, name="psu", tag="psu", bufs=2)
                        for h in range(NH):
                            nc.tensor.matmul(
                                psg, g16[:, h, :], xts16[h // 4][:, h % 4, g0:g1],
                                start=(h == 0), stop=(h == NH - 1),
                            )
                        for h in range(NH):
                            nc.tensor.matmul(
                                psu, u16[:, h, :], xts16[h // 4][:, h % 4, g0:g1],
                                start=(h == 0), stop=(h == NH - 1),
                            )
                        sil = tp.tile([128, wdt], f32, name="sil", tag="sil")
                        nc.scalar.activation(sil, psg, SILU)
                        nc.vector.tensor_mul(hm16[:, i, g0:g1], sil, psu)
                    for g0, g1 in ga8:
                        wdt = g1 - g0
                        psg8 = pp.tile([128, wdt], f32, name="psg8", tag="psg", bufs=3)
                        psu8 = pp.tile([128, wdt], f32, name="psu8", tag="psu", bufs=2)
                        for hh in range(NH // 2):
                            m = hh % 2
                            nc.tensor.matmul(
                                psg8,
                                g8[:, 2 * hh : 2 * hh + 2, :],
                                xts8[hh // 2][:, 2 * m : 2 * m + 2, g0:g1],
                                start=(hh == 0), stop=(hh == NH // 2 - 1),
                                perf_mode=DR,
                            )
                        for hh in range(NH // 2):
                            m = hh % 2
                            nc.tensor.matmul(
                                psu8,
                                u8[:, 2 * hh : 2 * hh + 2, :],
                                xts8[hh // 2][:, 2 * m : 2 * m + 2, g0:g1],
                                start=(hh == 0), stop=(hh == NH // 2 - 1),
                                perf_mode=DR,
                            )
                        sil8 = tp.tile([128, wdt], f32, name="sil8", tag="sil")
                        nc.scalar.activation(sil8, psg8, SILU, scale=1.0 / SG)
                        nc.vector.tensor_mul(hm8[:, i, g0:g1], sil8, psu8)
                for q in range(NH // 4):
                    ot16 = op.tile([128, 4, wa], f16, name="ot16", tag="ot16")
                    ot8 = op.tile([128, 4, wb], f16, name="ot8", tag="ot8")
                    for hq in range(4):
                        h = 4 * q + hq
                        for g0, g1 in ga16:
                            wdt = g1 - g0
                            pso = pp.tile([128, wdt], f32, name="pso", tag="pso", bufs=3)
                            for i in range(NI):
                                nc.tensor.matmul(
                                    pso, d16[:, h, i, :], hm16[:, i, g0:g1],
                                    start=(i == 0), stop=(i == NI - 1),
                                )
                            nc.vector.tensor_copy(ot16[:, hq, g0:g1], pso)
                        for g0, g1 in ga8:
                            wdt = g1 - g0
                            pso8 = pp.tile([128, wdt], f32, name="pso8", tag="pso", bufs=3)
                            for ii in range(NI // 2):
                                nc.tensor.matmul(
                                    pso8,
                                    d8[:, h, 2 * ii : 2 * ii + 2, :],
                                    hm8[:, 2 * ii : 2 * ii + 2, g0:g1],
                                    start=(ii == 0), stop=(ii == NI // 2 - 1),
                                    perf_mode=DR,
                                )
                            nc.scalar.activation(
                                ot8[:, hq, g0:g1], pso8,
                                mybir.ActivationFunctionType.Identity,
                            )
                    nc.gpsimd.dma_start(out=yT16s[k][q, :, :, :], in_=ot16)
                    nc.gpsimd.dma_start(out=yT8s[k][q, :, :, :], in_=ot8)
    nc.compile()
    return nc


def _install_neff_cache():
    """Cache walrus NEFF compiles on disk keyed by BIR hash (compile of the
    ~10k-instruction program takes minutes; the BIR is deterministic)."""
    import hashlib
    import shutil

    import concourse.bass2jax as bass2jax
    from concourse.bass_utils import compile_bir_kernel as _orig

    if getattr(bass2jax.compile_bir_kernel, "_moe_cached", False):
        return
    cache_dir = os.environ.get("BASS_NEFF_CACHE", "/tmp/bass_neff_cache")
    try:
        os.makedirs(cache_dir, exist_ok=True)
    except OSError:
        return

    def cached(bir_json, tmpdir, neff_name="file.neff"):
        key = hashlib.sha256(bir_json).hexdigest()[:24]
        cpath = os.path.join(cache_dir, key + ".neff")
        dst = os.path.join(tmpdir, neff_name)
        if os.path.exists(cpath):
            shutil.copy(cpath, dst)
            return dst
        out = _orig(bir_json, tmpdir, neff_name)
        try:
            shutil.copy(out, cpath)
        except OSError:
            pass
        return out

    cached._moe_cached = True
    bass2jax.compile_bir_kernel = cached


def _install_ntff_hook_shim():
    """Provide antenv.axon_hooks (absent in this container) so that
    run_bass_kernel_spmd(trace=True) can capture NTFF profiles via the
    axon .so — mirrors trn_agent_boot.trn_boot's ctypes hook."""
    import types
    import ctypes
    import contextlib

    if "antenv.axon_hooks" in sys.modules:
        return
    so_path = "/opt/axon/libaxon_pjrt.so"
    lib = ctypes.CDLL(so_path)
    if not hasattr(lib, "axon_start_nrt_profile"):
        return
    lib.axon_start_nrt_profile.argtypes = [
        ctypes.POINTER(ctypes.c_int64),
        ctypes.c_size_t,
    ]
    lib.axon_start_nrt_profile.restype = ctypes.c_int64
    lib.axon_stop_nrt_profile.argtypes = [ctypes.c_char_p]
    lib.axon_stop_nrt_profile.restype = ctypes.c_int64

    @contextlib.contextmanager
    def _hook(output_dir, device_ids):
        import jax

        jax.devices()
        if device_ids:
            ids = (ctypes.c_int64 * len(device_ids))(*device_ids)
            rc = lib.axon_start_nrt_profile(ids, len(device_ids))
        else:
            rc = lib.axon_start_nrt_profile(None, 0)
        if rc != 0:
            raise RuntimeError(f"axon_start_nrt_profile rc={rc}")
        try:
            yield
        finally:
            n = lib.axon_stop_nrt_profile(str(output_dir).encode())
            print(f"profile: {n} file(s) written to {output_dir}", file=sys.stderr)

    mod = types.ModuleType("antenv.axon_hooks")
    mod.get_axon_ntff_profile_hook = lambda: _hook
    mod.set_axon_ntff_profile_hook = lambda h: None
    sys.modules["antenv.axon_hooks"] = mod


def _assign_experts(n16, n8):
    """Assign experts to cores minimizing sum_j roundup8(max_c sorted16[c][j])
    + 0.5 * (same for fp8): LPT on weighted load, then swap refinement."""
    rng = np.random.default_rng(12345)
    wload = n16 + 0.5 * n8
    order = np.argsort(-wload, kind="stable")
    cores = [[] for _ in range(NCORES)]
    tot = np.zeros(NCORES)
    for e in order:
        c = int(np.argmin(tot))
        cores[c].append(int(e))
        tot[c] += wload[e]
    assign = np.zeros(E, dtype=np.int64)
    for c, ids in enumerate(cores):
        assign[ids] = c

    def cost(asg):
        m16 = np.zeros((NCORES, EPC))
        m8 = np.zeros((NCORES, EPC))
        for c in range(NCORES):
            ids = np.where(asg == c)[0]
            m16[c] = np.sort(n16[ids])[::-1]
            m8[c] = np.sort(n8[ids])[::-1]
        w16 = np.ceil(m16.max(0) / 8) * 8
        w8 = np.ceil(m8.max(0) / 8) * 8
        return w16.sum() + 0.5 * w8.sum()

    best = cost(assign)
    for _ in range(4000):
        e1, e2 = rng.integers(0, E, 2)
        c1, c2 = assign[e1], assign[e2]
        if c1 == c2:
            continue
        assign[e1], assign[e2] = c2, c1
        cnew = cost(assign)
        if cnew < best:
            best = cnew
        else:
            assign[e1], assign[e2] = c1, c2
    return assign


def kernel(
    hidden_states,
    routing_weights,
    selected_experts,
    gate_proj,
    up_proj,
    down_proj,
):
    global LAST_EXEC_NS, LAST_RESULTS
    from concourse.bass_utils import run_bass_kernel_spmd

    _install_neff_cache()

    x = np.ascontiguousarray(np.asarray(hidden_states, dtype=np.float32))
    rw = np.asarray(routing_weights, dtype=np.float32)
    sel = np.asarray(selected_experts).astype(np.int64)
    gate = np.asarray(gate_proj, dtype=np.float32)
    up = np.asarray(up_proj, dtype=np.float32)
    down = np.asarray(down_proj, dtype=np.float32)

    # ---- host dispatch (mirrors reference's stable sort-by-expert) ----
    flat_e = sel.reshape(-1)
    flat_p8 = rw.reshape(-1) < THETA  # fp8-path mask per (t, k) pair
    order = np.argsort(flat_e, kind="stable")
    sorted_e = flat_e[order]
    counts = np.bincount(flat_e, minlength=E)
    offsets = np.concatenate([[0], np.cumsum(counts)[:-1]])
    pos = np.arange(flat_e.shape[0], dtype=np.int64) - offsets[sorted_e]
    keep = pos < CAP  # reference capacity drop (pos within expert)

    p8_s = flat_p8[order]
    m16 = keep & ~p8_s
    m8 = keep & p8_s
    # rank within (expert, path), in stable order
    c16 = np.cumsum(m16)
    c8 = np.cumsum(m8)
    start16 = np.concatenate([[0], c16])[offsets[sorted_e]]
    start8 = np.concatenate([[0], c8])[offsets[sorted_e]]
    pos16 = c16 - 1 - start16  # valid where m16
    pos8 = c8 - 1 - start8
    n16 = np.bincount(sorted_e[m16], minlength=E)
    n8 = np.bincount(sorted_e[m8], minlength=E)

    # ---- expert -> core assignment + per-(path, slot) compile-time widths ----
    assign = _assign_experts(n16, n8)
    perm16 = np.zeros((NCORES, EPC), dtype=np.int64)
    perm8 = np.zeros((NCORES, EPC), dtype=np.int64)
    for c in range(NCORES):
        ids = np.where(assign == c)[0]
        perm16[c] = ids[np.argsort(-n16[ids], kind="stable")]
        perm8[c] = ids[np.argsort(-n8[ids], kind="stable")]
    w16s = tuple(
        int(min(CAP, max(16, -(-int(n16[perm16[:, j]].max()) // 8) * 8)))
        for j in range(EPC)
    )
    w8s = tuple(
        int(min(CAP, max(16, -(-int(n8[perm8[:, j]].max()) // 8) * 8)))
        for j in range(EPC)
    )

    tok = order // K
    maxw16, maxw8 = max(w16s), max(w8s)
    xbuf16 = np.zeros((E, maxw16, H), dtype=np.float32)
    xbuf16[sorted_e[m16], pos16[m16]] = x[tok[m16]]
    xbuf8 = np.zeros((E, maxw8, H), dtype=np.float32)
    xbuf8[sorted_e[m8], pos8[m8]] = x[tok[m8]]

    # ---- weight layouts (contiguous per-DMA blocks, shared fp16/fp8 shape) ----
    # gate/up slice (e, i): [128p, NH, 128c] where [p, h, c] = W[h*128+p, i*128+c]
    gate_r = gate.reshape(E, NH, 128, NI, 128).transpose(0, 3, 2, 1, 4)
    up_r = up.reshape(E, NH, 128, NI, 128).transpose(0, 3, 2, 1, 4)
    # down slice (e, h): [128p, NI, 128m] where [p, i, m] = W[i*128+p, h*128+m]
    down_r = down.reshape(E, NI, 128, NH, 128).transpose(0, 3, 2, 1, 4)
    gate16_full = np.ascontiguousarray(gate_r, dtype=np.float16)
    up16_full = np.ascontiguousarray(up_r, dtype=np.float16)
    down16_full = np.ascontiguousarray(down_r, dtype=np.float16)
    gate8_full = (gate_r * SG).astype(E4)
    up8_full = (up_r * SU).astype(E4)
    down8_full = (down_r * SD).astype(E4)

    key = (w16s, w8s)
    nc = _prog_cache.get(key)
    if nc is None:
        nc = _build_program(w16s, w8s)
        _prog_cache[key] = nc

    in_maps = []
    for c in range(NCORES):
        m = {
            "gw16": np.ascontiguousarray(gate16_full[perm16[c]]),
            "uw16": np.ascontiguousarray(up16_full[perm16[c]]),
            "dw16": np.ascontiguousarray(down16_full[perm16[c]]),
            "gw8": np.ascontiguousarray(gate8_full[perm8[c]]),
            "uw8": np.ascontiguousarray(up8_full[perm8[c]]),
            "dw8": np.ascontiguousarray(down8_full[perm8[c]]),
        }
        for j in range(EPC):
            e16, w16_ = perm16[c, j], w16s[j]
            e8, w8_ = perm8[c, j], w8s[j]
            m[f"x16_{j}"] = np.ascontiguousarray(
                xbuf16[e16, :w16_].T.reshape(NH, 128, w16_), dtype=np.float16
            )
            m[f"x8_{j}"] = np.ascontiguousarray(
                xbuf8[e8, :w8_].T.reshape(NH, 128, w8_)
            ).astype(E4)
        in_maps.append(m)

    trace = bool(os.environ.get("BASS_MOE_TRACE"))
    kwargs = {}
    if trace:
        _install_ntff_hook_shim()
        tcores = os.environ.get("BASS_MOE_TRACE_CORES", "0")
        kwargs = dict(trace=True, trace_cores=[int(c) for c in tcores.split(",")])
    res = run_bass_kernel_spmd(nc, in_maps, core_ids=list(range(NCORES)), **kwargs)
    LAST_EXEC_NS = res.exec_time_ns
    LAST_RESULTS = res

    # ---- host combine ----
    o16 = np.zeros((E, maxw16, H), dtype=np.float32)
    o8 = np.zeros((E, maxw8, H), dtype=np.float32)
    for c in range(NCORES):
        for j in range(EPC):
            # y arrays are [NH//4, 128p, 4hq, w]: (q, p, hq, n) = O^T[(4q+hq)*128+p, n]
            w = w16s[j]
            o16[perm16[c, j], :w] = (
                res.results[c][f"y16_{j}"]
                .astype(np.float32)
                .transpose(0, 2, 1, 3)
                .reshape(H, w)
                .T
            )
            w = w8s[j]
            o8[perm8[c, j], :w] = (
                res.results[c][f"y8_{j}"]
                .astype(np.float32)
                .transpose(0, 2, 1, 3)
                .reshape(H, w)
                .T
            )
    o8 *= 1.0 / (SU * SD)

    gathered = np.zeros((flat_e.shape[0], H), dtype=np.float32)
    gathered[order[m16]] = o16[sorted_e[m16], pos16[m16]]
    gathered[order[m8]] = o8[sorted_e[m8], pos8[m8]]
    y = (gathered.reshape(T, K, H) * rw[:, :, None]).sum(axis=1, dtype=np.float32)
    return y.astype(np.float32)
